# revision 3
# baseline (speedup 1.0000x reference)
"""Trainium2 Bass kernel: batched multi-head self-attention (nn_Attention).

y = softmax(q k^T / sqrt(64)) v, projected; x (8, 1025, 768), 12 heads x 64.

Strategy: batch-parallel across the 8 NeuronCores (one batch element per
core, no collectives). Per core, everything is kept feature-major
(transposed) so no on-chip transposes are needed:
  qkT = wqkT.T @ xT;  v = xT.T @ wvT (with a per-head ones column);
  scoresT = kT.T @ qT (keys on partitions, head pairs row-packed in the PE
  array);  exp on the scalar engine;  [v|1](128-wide).T @ attnT accumulated
  over key tiles yields the weighted values AND the softmax denominator in
  one PSUM accumulation;  normalize via reciprocal + gpsimd
  partition-broadcast;  yT = wpT.T @ aoT + bp.
Operands are fp16 (inputs/weights/q/k/v, ~2e-3 relative accuracy) except the
exp'd attention weights, which are bf16 (exp reaches ~5e6, beyond fp16
range); all accumulation is fp32 in PSUM. Full-array "warmer" matmuls keep
the PE HAM clock at 8/8 through the partial-array attention matmuls.
"""
import sys

try:
    import concourse.bass  # noqa: F401
except ImportError:
    sys.path.insert(0, "/opt/trn_rl_repo")

import numpy as np

from contextlib import ExitStack

import concourse.bass as bass
import concourse.tile as tile
from concourse import bacc, mybir

F32 = mybir.dt.float32
F32R = mybir.dt.float32r
BF16 = mybir.dt.bfloat16

C = 768
H = 12
D = 64
NTOK = 1025
T = 1032
CT = C // 128
SCALE = D ** -0.5

KT = [(i * 128, 128) for i in range(8)] + [(1024, 8)]
QC = [(0, 512), (512, 512), (1024, 8)]
VC = [(0, 512), (512, 256)]
SC_GROUPS = [(0, 1), (2, 3), (4, 5), (6, 7), (8,)]
VW = 65

B_OT_ORDER = [0, 6, 1, 7, 2, 8, 3, 9, 4, 10, 5, 11]


F16 = mybir.dt.float16


def build(matmul_dtype="fp16"):
    # MT: projection operands (x, weights, aoT). AT: q/k/v storage.
    # ATTN: exp output / AV moving operand (bf16: exp can reach ~5e6,
    # which overflows fp16).
    if matmul_dtype == "bf16":
        MT = AT = ATTN = BF16
    elif matmul_dtype == "fp16":
        MT = AT = F16
        ATTN = BF16
    elif matmul_dtype == "f32":
        MT = AT = ATTN = F32
    elif matmul_dtype == "bf16attn":
        MT = F32R
        AT = ATTN = BF16
    else:
        MT = AT = ATTN = F32R
    nc = bacc.Bacc("TRN2", target_bir_lowering=False, debug=False, num_devices=8)

    xT_d = nc.dram_tensor("xT", [C, T], MT, kind="ExternalInput")
    wqkT_d = nc.dram_tensor("wqkT", [C, 2 * C], MT, kind="ExternalInput")
    wvT_d = nc.dram_tensor("wvT", [C, C], MT, kind="ExternalInput")
    wpT_d = nc.dram_tensor("wpT", [C, C], MT, kind="ExternalInput")
    bp_d = nc.dram_tensor("bp", [C, 1], F32, kind="ExternalInput")
    yT_d = nc.dram_tensor("yT", [C, T], F16 if matmul_dtype == "fp16" else F32, kind="ExternalOutput")

    with tile.TileContext(nc) as tc, ExitStack() as ctx:
        p_qk = ctx.enter_context(tc.tile_pool(name="qk", bufs=1))
        p_v = ctx.enter_context(tc.tile_pool(name="v", bufs=1))
        p_ao = ctx.enter_context(tc.tile_pool(name="ao", bufs=1))
        p_bp = ctx.enter_context(tc.tile_pool(name="bp", bufs=1))
        p_attn = ctx.enter_context(tc.tile_pool(name="attn", bufs=1))
        p_sm = ctx.enter_context(tc.tile_pool(name="sm", bufs=6))
        p_stage = ctx.enter_context(tc.tile_pool(name="stage", bufs=4))
        p_wp = ctx.enter_context(tc.tile_pool(name="wp", bufs=1))

        qkT = [p_qk.tile([128, T], AT, tag=f"qkT{i}", name=f"qkT{i}") for i in range(12)]
        v_ext = [p_v.tile([128, H * VW + 63], AT, tag=f"v{i}", name=f"v{i}") for i in range(9)]
        aoT = [p_ao.tile([128, T], MT, tag=f"ao{i}", name=f"ao{i}") for i in range(CT)]
        bp_sb = [p_bp.tile([128, 1], F32, tag=f"bp{i}", name=f"bp{i}") for i in range(CT)]

        with tc.tile_pool(name="x", bufs=1) as p_x, \
             tc.tile_pool(name="wv", bufs=1) as p_wv, \
             tc.tile_pool(name="psBC", bufs=3, space="PSUM") as psBC:
            xT = [p_x.tile([128, T], MT, tag=f"x{i}", name=f"x{i}") for i in range(CT)]
            wvT = [p_wv.tile([128, C], MT, tag=f"wv{i}", name=f"wv{i}") for i in range(CT)]
            for c in range(CT):
                nc.sync.dma_start(xT[c][:], xT_d.ap()[c * 128:(c + 1) * 128, :])
                nc.sync.dma_start(wvT[c][:], wvT_d.ap()[c * 128:(c + 1) * 128, :])
            wqk_resident = mybir.dt.size(MT) == 2
            if wqk_resident:
                wqk = [p_wv.tile([128, 2 * C], MT, tag=f"wqk{i}", name=f"wqk{i}")
                       for i in range(CT)]
                for c in range(CT):
                    nc.sync.dma_start(wqk[c][:], wqkT_d.ap()[c * 128:(c + 1) * 128, :])

            def wqk_slice(c, ot):
                if wqk_resident:
                    return wqk[c][:, ot * 128:(ot + 1) * 128]
                t = p_wv.tile([128, 128], MT, tag="wqks", name="wqks", bufs=18)
                nc.sync.dma_start(
                    t[:], wqkT_d.ap()[c * 128:(c + 1) * 128, ot * 128:(ot + 1) * 128])
                return t[:]

            # ---- phase C ----
            with nc.named_scope("v_proj"):
                for nt, (noff, nsz) in enumerate(KT):
                    for (voff, vsz) in VC:
                        ps = psBC.tile([128, 512], F32, tag="proj", name="ps_proj")
                        for c in range(CT):
                            nc.tensor.matmul(
                                ps[:nsz, :vsz],
                                xT[c][:, noff:noff + nsz],
                                wvT[c][:, voff:voff + vsz],
                                start=(c == 0), stop=(c == CT - 1),
                            )
                        nh = vsz // D
                        h0 = voff // D
                        dst = (
                            v_ext[nt][0:nsz, h0 * VW:(h0 + nh) * VW]
                            .rearrange("p (hh w) -> p hh w", w=VW)[:, :, 0:D]
                        )
                        src = ps[0:nsz, 0:vsz].rearrange("p (hh w) -> p hh w", w=D)
                        nc.vector.tensor_copy(dst, src)
                    # ones column (valid tokens only) + zeroed pad/tail
                    if nt < 8:
                        ones_col = (
                            v_ext[nt][0:nsz, 0:H * VW]
                            .rearrange("p (hh w) -> p hh w", w=VW)[:, :, D:VW]
                        )
                        _memset(nc, AT, ones_col, one=True)
                    else:
                        pad_col = (
                            v_ext[nt][0:nsz, 0:H * VW]
                            .rearrange("p (hh w) -> p hh w", w=VW)[:, :, D:VW]
                        )
                        _memset(nc, AT, pad_col, one=False)
                        one_row = (
                            v_ext[nt][0:1, 0:H * VW]
                            .rearrange("p (hh w) -> p hh w", w=VW)[:, :, D:VW]
                        )
                        _memset(nc, AT, one_row, one=True)
                for nt in range(9):
                    _memset(nc, AT, v_ext[nt][:, H * VW:H * VW + 63], one=False)

            # ---- phase B ----
            with nc.named_scope("qk_proj"):
                for ot in B_OT_ORDER:
                    wslices = [wqk_slice(c, ot) for c in range(CT)]
                    for (qoff, qsz) in QC[:2]:
                        ps = psBC.tile([128, 512], F32, tag="proj", name="ps_proj")
                        for c in range(CT):
                            nc.tensor.matmul(
                                ps[:, :qsz],
                                wslices[c],
                                xT[c][:, qoff:qoff + qsz],
                                start=(c == 0), stop=(c == CT - 1),
                            )
                        nc.vector.tensor_copy(qkT[ot][:, qoff:qoff + qsz], ps[:, :qsz])
                # 8-col tails, two otiles per psum slot
                qoff, qsz = QC[2]
                for pair_ot in range(CT):
                    ps = psBC.tile([128, 512], F32, tag="proj", name="ps_proj")
                    for oi, ot in enumerate((pair_ot, 6 + pair_ot)):
                        wslices = [wqk_slice(c, ot) for c in range(CT)]
                        for c in range(CT):
                            nc.tensor.matmul(
                                ps[:, oi * 8:oi * 8 + qsz],
                                wslices[c],
                                xT[c][:, qoff:qoff + qsz],
                                start=(c == 0), stop=(c == CT - 1),
                            )
                    nc.vector.tensor_copy(qkT[pair_ot][:, qoff:qoff + qsz], ps[:, 0:qsz])
                    nc.vector.tensor_copy(qkT[6 + pair_ot][:, qoff:qoff + qsz],
                                          ps[:, 8:8 + qsz])

        # ---- phase D ----
        with nc.named_scope("attn"), \
             tc.tile_pool(name="psD", bufs=1, space="PSUM") as psD:
            _emit_attention(nc, tc, qkT, v_ext, aoT, psD, p_attn, p_sm, ATTN)

        # ---- phase E ----
        with nc.named_scope("out_proj"), \
             tc.tile_pool(name="psE", bufs=3, space="PSUM") as psE:
            wp = [p_wp.tile([128, C], MT, tag=f"wp{i}", name=f"wp{i}") for i in range(CT)]
            for c in range(CT):
                nc.sync.dma_start(wp[c][:], wpT_d.ap()[c * 128:(c + 1) * 128, :])
                nc.sync.dma_start(bp_sb[c][:], bp_d.ap()[c * 128:(c + 1) * 128, :])
            def e_chunk(qoff, qsz):
                for ot in range(CT):
                    ps = psE.tile([128, 512], F32, tag="proj", name="ps_proj")
                    for c in range(CT):
                        nc.tensor.matmul(
                            ps[:, :qsz],
                            wp[c][:, ot * 128:(ot + 1) * 128],
                            aoT[c][:, qoff:qoff + qsz],
                            start=(c == 0), stop=(c == CT - 1),
                        )
                    st = p_stage.tile([128, 512], F16 if MT == F16 else F32, tag="ystage", name="ystage")
                    nc.vector.tensor_scalar_add(st[:, :qsz], ps[:, :qsz], bp_sb[ot][:, 0:1])
                    nc.sync.dma_start(yT_d.ap()[ot * 128:(ot + 1) * 128, qoff:qoff + qsz],
                                      st[:, :qsz])

            # qc2-attention early in the tail: its exp/normalize (ACT/DVE)
            # drains under the remaining E matmul streams
            e_chunk(*QC[0])
            _emit_qc2_attn(nc, qkT, v_ext, aoT, psE, p_attn, p_sm, ATTN)
            e_chunk(*QC[1])
            e_chunk(*QC[2])

    nc.compile()
    return nc


def _memset(nc, AT, ap, one):
    if AT == BF16:
        nc.vector.memset(ap.bitcast(mybir.dt.uint16), 0x3F80 if one else 0)
    elif AT == F16:
        nc.vector.memset(ap.bitcast(mybir.dt.uint16), 0x3C00 if one else 0)
    else:
        nc.vector.memset(ap.bitcast(mybir.dt.uint32), 0x3F800000 if one else 0)


def _emit_attention(nc, tc, qkT, v_ext, aoT, psD, p_attn, p_sm, ATTN):
    def scores_mm(sc, pair, h_in_pair, kt, dst_off, qoff, qsz):
        koff, ksz = KT[kt]
        p0 = 64 * h_in_pair
        nc.tensor.matmul(
            sc[0:ksz, dst_off:dst_off + qsz],
            qkT[6 + pair][p0:p0 + 64, koff:koff + ksz],
            qkT[pair][p0:p0 + 64, qoff:qoff + qsz],
            start=True, stop=True,
        )

    def av_mm(av, h, kt, src, src_off, qsz):
        koff, ksz = KT[kt]
        nc.tensor.matmul(
            av[0:128, 0:qsz],
            v_ext[kt][0:ksz, h * VW:h * VW + 128],
            src[0:ksz, src_off:src_off + qsz],
            start=(kt == 0), stop=(kt == 8),
            skip_group_check=True,
        )

    def normalize(avs, pair, h_in_pair, qoff, qsz):
        h = 2 * pair + h_in_pair
        p0 = 64 * h_in_pair
        av_sb = p_sm.tile([128, 512], F32, tag="avsb", name="avsb")
        nc.vector.tensor_copy(av_sb[0:VW, 0:qsz], avs[h][0:VW, 0:qsz])
        rec = p_sm.tile([1, 512], F32, tag="rec", name="rec")
        nc.vector.reciprocal(rec[0:1, 0:qsz], av_sb[D:VW, 0:qsz])
        bc = p_sm.tile([64, 512], F32, tag="bc", name="bc")
        nc.gpsimd.partition_broadcast(bc[0:64, 0:qsz], rec[0:1, 0:qsz])
        nc.vector.tensor_mul(
            aoT[pair][p0:p0 + 64, qoff:qoff + qsz],
            av_sb[0:D, 0:qsz],
            bc[0:64, 0:qsz],
        )

    # big query chunks, qc-major: the out-projection for a chunk unblocks
    # after the last pair finishes it, early in the attention window
    for (qoff, qsz) in QC[:2]:
        for pair in range(6):
            h0 = 2 * pair
            h1 = 2 * pair + 1
            avs = {h0: psD.tile([128, 512], F32, tag="av", name="ps_av", bufs=2),
                   h1: psD.tile([128, 512], F32, tag="av", name="ps_av", bufs=2)}
            for g in SC_GROUPS:
                sc_slots = {}
                for h_in_pair in range(2):
                    sc_slots[h_in_pair] = psD.tile([128, 1024], F32, tag="sc",
                                                    name="ps_sc", bufs=3)
                # full-array warmer; overwritten by the scores below
                nc.tensor.matmul(
                    sc_slots[0][:, 0:512],
                    qkT[pair][:, 0:128],
                    qkT[pair][:, 0:512],
                    start=True, stop=True,
                )
                for gi, kt in enumerate(g):
                    for h_in_pair in range(2):
                        scores_mm(sc_slots[h_in_pair], pair, h_in_pair, kt,
                                  gi * 512, qoff, qsz)
                at = {}
                for h_in_pair in range(2):
                    h = 2 * pair + h_in_pair
                    a = p_attn.tile([128, 1024], ATTN, tag="attnT", name="attnT", bufs=(8 if ATTN == BF16 else 3))
                    at[h] = a
                    width = len(g) * 512
                    pmax = max(KT[kt][1] for kt in g)
                    nc.scalar.activation(
                        a[0:pmax, 0:width], sc_slots[h_in_pair][0:pmax, 0:width],
                        mybir.ActivationFunctionType.Exp, scale=SCALE,
                    )
                for gi, kt in enumerate(g):
                    av_mm(avs[h0], h0, kt, at[h0], gi * 512, qsz)
                    av_mm(avs[h1], h1, kt, at[h1], gi * 512, qsz)
            normalize(avs, pair, 0, qoff, qsz)
            normalize(avs, pair, 1, qoff, qsz)


def _emit_qc2_attn(nc, qkT, v_ext, aoT, psE, p_attn, p_sm, ATTN):
    def scores_mm(sc, pair, h_in_pair, kt, dst_off, qoff, qsz):
        koff, ksz = KT[kt]
        p0 = 64 * h_in_pair
        nc.tensor.matmul(
            sc[0:ksz, dst_off:dst_off + qsz],
            qkT[6 + pair][p0:p0 + 64, koff:koff + ksz],
            qkT[pair][p0:p0 + 64, qoff:qoff + qsz],
            start=True, stop=True,
        )

    def av_mm(av, h, kt, src, src_off, qsz):
        koff, ksz = KT[kt]
        nc.tensor.matmul(
            av[0:128, 0:qsz],
            v_ext[kt][0:ksz, h * VW:h * VW + 128],
            src[0:ksz, src_off:src_off + qsz],
            start=(kt == 0), stop=(kt == 8),
            skip_group_check=True,
        )

    def normalize(avs, pair, h_in_pair, qoff, qsz):
        h = 2 * pair + h_in_pair
        p0 = 64 * h_in_pair
        av_sb = p_sm.tile([128, 512], F32, tag="avsb", name="avsb")
        nc.vector.tensor_copy(av_sb[0:VW, 0:qsz], avs[h][0:VW, 0:qsz])
        rec = p_sm.tile([1, 512], F32, tag="rec", name="rec")
        nc.vector.reciprocal(rec[0:1, 0:qsz], av_sb[D:VW, 0:qsz])
        bc = p_sm.tile([64, 512], F32, tag="bc", name="bc")
        nc.gpsimd.partition_broadcast(bc[0:64, 0:qsz], rec[0:1, 0:qsz])
        nc.vector.tensor_mul(
            aoT[pair][p0:p0 + 64, qoff:qoff + qsz],
            av_sb[0:D, 0:qsz],
            bc[0:64, 0:qsz],
        )

    qoff, qsz = QC[2]
    for pair in range(6):
        for h_in_pair in range(2):
            h = 2 * pair + h_in_pair
            sc = psE.tile([128, 512], F32, tag="proj", name="ps_proj")
            for kt in range(9):
                scores_mm(sc, pair, h_in_pair, kt, kt * 8, qoff, qsz)
            a = p_attn.tile([128, 1024], ATTN, tag="attnT", name="attnT", bufs=(8 if ATTN == BF16 else 3))
            nc.scalar.activation(
                a[:, 0:64], sc[:, 0:64],
                mybir.ActivationFunctionType.Exp, scale=SCALE,
            )
            nc.scalar.activation(
                a[0:8, 64:72], sc[0:8, 64:72],
                mybir.ActivationFunctionType.Exp, scale=SCALE,
            )
            av = psE.tile([128, 512], F32, tag="proj", name="ps_proj")
            for kt in range(9):
                av_mm(av, h, kt, a, kt * 8, qsz)
            normalize({h: av}, pair, h_in_pair, qoff, qsz)


_NC_CACHE = {}
_MODE = "fp16"
LAST_RESULT = None


def kernel(x, w_qkv, w_proj, b_proj, *, _trace=False, _tmpdir=None):
    x = np.asarray(x, np.float32)
    w_qkv = np.asarray(w_qkv, np.float32)
    w_proj = np.asarray(w_proj, np.float32)
    b_proj = np.asarray(b_proj, np.float32)
    B = x.shape[0]
    assert x.shape == (8, NTOK, C), x.shape

    mt = np.float16 if _MODE == "fp16" else np.float32
    wqkT = np.ascontiguousarray(w_qkv[:2 * C].T.astype(mt))
    wvT = np.ascontiguousarray(w_qkv[2 * C:].T.astype(mt))
    wpT = np.ascontiguousarray(w_proj.T.astype(mt))
    bp = np.ascontiguousarray(b_proj.reshape(C, 1))
    in_maps = []
    for b in range(B):
        xT = np.zeros((C, T), mt)
        xT[:, :NTOK] = x[b].T.astype(mt)
        in_maps.append({"xT": xT, "wqkT": wqkT, "wvT": wvT, "wpT": wpT, "bp": bp})

    if _MODE not in _NC_CACHE:
        _NC_CACHE[_MODE] = build(matmul_dtype=_MODE)
    nc = _NC_CACHE[_MODE]
    from concourse import bass_utils
    res = bass_utils.run_bass_kernel_spmd(nc, in_maps, core_ids=list(range(B)),
                                          trace=_trace, tmpdir=_tmpdir)
    global LAST_RESULT
    LAST_RESULT = res
    y = np.stack([res.results[b]["yT"][:, :NTOK].T for b in range(B)])
    return np.ascontiguousarray(y.astype(np.float32))



# revision 6
# speedup vs baseline: 1.0886x; 1.0886x over previous
"""Trainium2 Bass kernel: batched multi-head self-attention (nn_Attention).

y = softmax(q k^T / sqrt(64)) v, projected; x (8, 1025, 768), 12 heads x 64.

Strategy: batch-parallel across the 8 NeuronCores (one batch element per
core, no collectives). Per core, everything is kept feature-major
(transposed) so no on-chip transposes are needed:
  qkT = wqkT.T @ xT;  v = xT.T @ wvT (with a per-head ones column);
  scoresT = kT.T @ qpad (keys on partitions; q zero-padded per head so the
  K stationary is the full 128-row head pair — full PE rows, no HAM droop);
  exp on the scalar engine;  [v|1](128-wide).T @ attnT accumulated over key
  tiles yields the weighted values AND the softmax denominator in one PSUM
  accumulation;  normalize via fast-approx reciprocal + gpsimd
  partition-broadcast;  yT = wpT.T @ aoT + bp.
Queries are processed in three 344-column chunks (no scalar tail pass).
Operands are fp16 (inputs/weights/q/k/v, ~2e-3 relative accuracy) except the
exp'd attention weights, which are bf16 (exp reaches ~5e6, beyond fp16
range); all accumulation is fp32 in PSUM.
"""
import sys

try:
    import concourse.bass  # noqa: F401
except ImportError:
    sys.path.insert(0, "/opt/trn_rl_repo")

import numpy as np

from contextlib import ExitStack

import concourse.bass as bass
import concourse.tile as tile
from concourse import bacc, mybir

F32 = mybir.dt.float32
F32R = mybir.dt.float32r
BF16 = mybir.dt.bfloat16
F16 = mybir.dt.float16

C = 768
H = 12
D = 64
NTOK = 1025
T = 1032
CT = C // 128
SCALE = D ** -0.5

KT = [(i * 128, 128) for i in range(8)] + [(1024, 8)]
QC = [(0, 344), (344, 344), (688, 344)]
PC = [(0, 512), (512, 512), (1024, 8)]
SC_GROUPS = [(0, 1), (2, 3), (4, 5), (6, 7), (8,)]
VW = 65


def build(matmul_dtype="fp16"):
    # MT: projection operands (x, weights, aoT). AT: q/k/v storage.
    # ATTN: exp output / AV moving operand (bf16: exp can reach ~5e6,
    # which overflows fp16).
    if matmul_dtype == "fp16":
        MT = AT = F16
        ATTN = BF16
    elif matmul_dtype == "bf16":
        MT = AT = ATTN = BF16
    else:
        MT = AT = ATTN = F32
    nc = bacc.Bacc("TRN2", target_bir_lowering=False, debug=False, num_devices=8)

    xT_d = nc.dram_tensor("xT", [C, T], MT, kind="ExternalInput")
    wqkT_d = nc.dram_tensor("wqkT", [C, 2 * C], MT, kind="ExternalInput")
    wvT_d = nc.dram_tensor("wvT", [C, C], MT, kind="ExternalInput")
    wpT_d = nc.dram_tensor("wpT", [C, C], MT, kind="ExternalInput")
    bp_d = nc.dram_tensor("bp", [C, 1], F32, kind="ExternalInput")
    yT_d = nc.dram_tensor("yT", [C, T], F16 if matmul_dtype == "fp16" else F32,
                          kind="ExternalOutput")

    with tile.TileContext(nc) as tc, ExitStack() as ctx:
        p_k = ctx.enter_context(tc.tile_pool(name="k", bufs=1))
        p_qp = ctx.enter_context(tc.tile_pool(name="qp", bufs=1))
        p_v = ctx.enter_context(tc.tile_pool(name="v", bufs=1))
        p_ao = ctx.enter_context(tc.tile_pool(name="ao", bufs=1))
        p_bp = ctx.enter_context(tc.tile_pool(name="bp", bufs=1))
        p_attn = ctx.enter_context(tc.tile_pool(name="attn", bufs=1))
        p_sm = ctx.enter_context(tc.tile_pool(name="sm", bufs=6))
        p_stage = ctx.enter_context(tc.tile_pool(name="stage", bufs=3))
        p_wp = ctx.enter_context(tc.tile_pool(name="wp", bufs=1))

        # kT tiles: rows = both heads of a pair (128); qpad: per pair,
        # [128, 2T]: cols 0:T = h0's q (rows 64:128 zero), cols T:2T = h1's
        # q (rows 0:64 zero) -> scores use the full-width K stationary.
        kT = [p_k.tile([128, T], AT, tag=f"kT{i}", name=f"kT{i}") for i in range(6)]
        qpad = [p_qp.tile([128, 2 * T], AT, tag=f"qp{i}", name=f"qp{i}")
                for i in range(6)]
        v_ext = [p_v.tile([128, H * VW + 63], AT, tag=f"v{i}", name=f"v{i}")
                 for i in range(9)]
        aoT = [p_ao.tile([128, T], MT, tag=f"ao{i}", name=f"ao{i}") for i in range(CT)]
        bp_sb = [p_bp.tile([128, 1], F32, tag=f"bp{i}", name=f"bp{i}")
                 for i in range(CT)]
        wp = [p_wp.tile([128, C], MT, tag=f"wp{i}", name=f"wp{i}") for i in range(CT)]

        with tc.tile_pool(name="x", bufs=1) as p_x, \
             tc.tile_pool(name="wv", bufs=1) as p_wv, \
             tc.tile_pool(name="psBC", bufs=6, space="PSUM") as psBC:
            xT = [p_x.tile([128, T], MT, tag=f"x{i}", name=f"x{i}") for i in range(CT)]
            wvT = [p_wv.tile([128, C], MT, tag=f"wv{i}", name=f"wv{i}")
                   for i in range(CT)]
            wqk = [p_wv.tile([128, 2 * C], MT, tag=f"wqk{i}", name=f"wqk{i}")
                   for i in range(CT)]
            for c in range(CT):
                nc.sync.dma_start(xT[c][:], xT_d.ap()[c * 128:(c + 1) * 128, :])
                nc.sync.dma_start(wvT[c][:], wvT_d.ap()[c * 128:(c + 1) * 128, :])
                nc.sync.dma_start(wqk[c][:], wqkT_d.ap()[c * 128:(c + 1) * 128, :])
                nc.sync.dma_start(wp[c][:], wpT_d.ap()[c * 128:(c + 1) * 128, :])
                nc.sync.dma_start(bp_sb[c][:], bp_d.ap()[c * 128:(c + 1) * 128, :])

            # zero the cross-head quadrants of qpad (pad tokens come out
            # zero via the zeroed xT pad columns)
            for pair in range(6):
                _memset(nc, AT, qpad[pair][64:128, 0:T], one=False)
                _memset(nc, AT, qpad[pair][0:64, T:2 * T], one=False)

            # ---- phase C: v = xT.T @ wvT (token-major) ----
            with nc.named_scope("v_proj"):
                for nt, (noff, nsz) in enumerate(KT):
                    psa = psBC.tile([128, 512], F32, tag="proj", name="ps_proj")
                    psb = psBC.tile([128, 512], F32, tag="proj", name="ps_proj")
                    for c in range(CT):
                        xs = xT[c][:, noff:noff + nsz]
                        nc.tensor.matmul(psa[:nsz, :512], xs, wvT[c][:, 0:512],
                                         start=(c == 0), stop=(c == CT - 1))
                        nc.tensor.matmul(psb[:nsz, :256], xs, wvT[c][:, 512:768],
                                         start=(c == 0), stop=(c == CT - 1))
                    for (voff, vsz), ps in (((0, 512), psa), ((512, 256), psb)):
                        nh = vsz // D
                        h0 = voff // D
                        dst = (
                            v_ext[nt][0:nsz, h0 * VW:(h0 + nh) * VW]
                            .rearrange("p (hh w) -> p hh w", w=VW)[:, :, 0:D]
                        )
                        src = ps[0:nsz, 0:vsz].rearrange("p (hh w) -> p hh w", w=D)
                        nc.vector.tensor_copy(dst, src)
                    # ones column (valid tokens only) + zeroed pad/tail
                    if nt < 8:
                        ones_col = (
                            v_ext[nt][0:nsz, 0:H * VW]
                            .rearrange("p (hh w) -> p hh w", w=VW)[:, :, D:VW]
                        )
                        _memset(nc, AT, ones_col, one=True)
                    else:
                        pad_col = (
                            v_ext[nt][0:nsz, 0:H * VW]
                            .rearrange("p (hh w) -> p hh w", w=VW)[:, :, D:VW]
                        )
                        _memset(nc, AT, pad_col, one=False)
                        one_row = (
                            v_ext[nt][0:1, 0:H * VW]
                            .rearrange("p (hh w) -> p hh w", w=VW)[:, :, D:VW]
                        )
                        _memset(nc, AT, one_row, one=True)
                for nt in range(9):
                    _memset(nc, AT, v_ext[nt][:, H * VW:H * VW + 63], one=False)

            # ---- phase B: q/k projections ----
            with nc.named_scope("qk_proj"):
                for ot in range(12):
                    pss = [psBC.tile([128, 512], F32, tag="proj", name="ps_proj")
                           for _ in range(3)]
                    for c in range(CT):
                        w = wqk[c][:, ot * 128:(ot + 1) * 128]
                        for (qoff, qsz), ps in zip(PC, pss):
                            nc.tensor.matmul(ps[:, :qsz], w,
                                             xT[c][:, qoff:qoff + qsz],
                                             start=(c == 0), stop=(c == CT - 1))
                    if ot < 6:
                        # q: split the two heads into zero-padded halves
                        for (qoff, qsz), ps in zip(PC, pss):
                            nc.vector.tensor_copy(
                                qpad[ot][0:64, qoff:qoff + qsz], ps[0:64, :qsz])
                            nc.vector.tensor_copy(
                                qpad[ot][64:128, T + qoff:T + qoff + qsz],
                                ps[64:128, :qsz])
                    else:
                        for (qoff, qsz), ps in zip(PC, pss):
                            nc.vector.tensor_copy(
                                kT[ot - 6][:, qoff:qoff + qsz], ps[:, :qsz])

        # ---- phase D: attention ----
        with nc.named_scope("attn"), \
             tc.tile_pool(name="psD", bufs=1, space="PSUM") as psD:
            _emit_attention(nc, tc, kT, qpad, v_ext, aoT, psD, p_attn, p_sm, ATTN)

        # ---- phase E: out projection ----
        with nc.named_scope("out_proj"), \
             tc.tile_pool(name="psE", bufs=6, space="PSUM") as psE:
            for ot in range(CT):
                pss = [psE.tile([128, 512], F32, tag="proj", name="ps_proj")
                       for _ in range(3)]
                for c in range(CT):
                    w = wp[c][:, ot * 128:(ot + 1) * 128]
                    for (qoff, qsz), ps in zip(PC, pss):
                        nc.tensor.matmul(ps[:, :qsz], w,
                                         aoT[c][:, qoff:qoff + qsz],
                                         start=(c == 0), stop=(c == CT - 1))
                st = p_stage.tile([128, T], F16 if MT == F16 else F32,
                                  tag="ystage", name="ystage")
                for (qoff, qsz), ps in zip(PC, pss):
                    nc.vector.tensor_scalar_add(st[:, qoff:qoff + qsz],
                                                ps[:, :qsz], bp_sb[ot][:, 0:1])
                nc.sync.dma_start(yT_d.ap()[ot * 128:(ot + 1) * 128, :], st[:])

    nc.compile()
    return nc


def _memset(nc, AT, ap, one):
    if AT == BF16:
        nc.vector.memset(ap.bitcast(mybir.dt.uint16), 0x3F80 if one else 0)
    elif AT == F16:
        nc.vector.memset(ap.bitcast(mybir.dt.uint16), 0x3C00 if one else 0)
    else:
        nc.vector.memset(ap.bitcast(mybir.dt.uint32), 0x3F800000 if one else 0)


def _emit_attention(nc, tc, kT, qpad, v_ext, aoT, psD, p_attn, p_sm, ATTN):
    def scores_mm(sc, pair, h_in_pair, kt, dst_off, qoff, qsz):
        koff, ksz = KT[kt]
        nc.tensor.matmul(
            sc[0:ksz, dst_off:dst_off + qsz],
            kT[pair][:, koff:koff + ksz],
            qpad[pair][:, h_in_pair * T + qoff:h_in_pair * T + qoff + qsz],
            start=True, stop=True,
        )

    def av_mm(av, h, kt, src, src_off, qsz):
        koff, ksz = KT[kt]
        nc.tensor.matmul(
            av[0:128, 0:qsz],
            v_ext[kt][0:ksz, h * VW:h * VW + 128],
            src[0:ksz, src_off:src_off + qsz],
            start=(kt == 0), stop=(kt == 8),
            skip_group_check=True,
        )

    def normalize(avs, pair, h_in_pair, qoff, qsz):
        h = 2 * pair + h_in_pair
        p0 = 64 * h_in_pair
        av_sb = p_sm.tile([128, 512], F32, tag="avsb", name="avsb")
        nc.vector.tensor_copy(av_sb[0:VW, 0:qsz], avs[h][0:VW, 0:qsz])
        rec = p_sm.tile([1, 512], F32, tag="rec", name="rec")
        nc.vector.reciprocal(rec[0:1, 0:qsz], av_sb[D:VW, 0:qsz])
        bc = p_sm.tile([64, 512], F32, tag="bc", name="bc")
        nc.gpsimd.partition_broadcast(bc[0:64, 0:qsz], rec[0:1, 0:qsz])
        nc.vector.tensor_mul(
            aoT[pair][p0:p0 + 64, qoff:qoff + qsz],
            av_sb[0:D, 0:qsz],
            bc[0:64, 0:qsz],
        )

    for (qoff, qsz) in QC:
        for pair in range(6):
            h0 = 2 * pair
            h1 = 2 * pair + 1
            avs = {h0: psD.tile([128, 512], F32, tag="av", name="ps_av", bufs=2),
                   h1: psD.tile([128, 512], F32, tag="av", name="ps_av", bufs=2)}
            for g in SC_GROUPS:
                sc_slots = {}
                for h_in_pair in range(2):
                    sc_slots[h_in_pair] = psD.tile([128, 1024], F32, tag="sc",
                                                   name="ps_sc", bufs=3)
                for gi, kt in enumerate(g):
                    for h_in_pair in range(2):
                        scores_mm(sc_slots[h_in_pair], pair, h_in_pair, kt,
                                  gi * 512, qoff, qsz)
                at = {}
                for h_in_pair in range(2):
                    h = 2 * pair + h_in_pair
                    a = p_attn.tile([128, 1024], ATTN, tag="attnT", name="attnT",
                                    bufs=8)
                    at[h] = a
                    pmax = max(KT[kt][1] for kt in g)
                    if len(g) == 2:
                        # src: the two kt blocks sit at bank-aligned cols
                        # 0 and 512; dst is packed at stride qsz
                        src = sc_slots[h_in_pair][0:pmax, 0:1024] \
                            .rearrange("p (g w) -> p g w", w=512)[:, :, 0:qsz]
                        dst = a[0:pmax, 0:2 * qsz] \
                            .rearrange("p (g w) -> p g w", w=qsz)
                    else:
                        src = sc_slots[h_in_pair][0:pmax, 0:qsz]
                        dst = a[0:pmax, 0:qsz]
                    nc.scalar.activation(
                        dst, src, mybir.ActivationFunctionType.Exp, scale=SCALE,
                    )
                for gi, kt in enumerate(g):
                    av_mm(avs[h0], h0, kt, at[h0], gi * qsz, qsz)
                    av_mm(avs[h1], h1, kt, at[h1], gi * qsz, qsz)
            normalize(avs, pair, 0, qoff, qsz)
            normalize(avs, pair, 1, qoff, qsz)


_NC_CACHE = {}
_MODE = "fp16"
LAST_RESULT = None


def kernel(x, w_qkv, w_proj, b_proj, *, _trace=False, _tmpdir=None):
    x = np.asarray(x, np.float32)
    w_qkv = np.asarray(w_qkv, np.float32)
    w_proj = np.asarray(w_proj, np.float32)
    b_proj = np.asarray(b_proj, np.float32)
    B = x.shape[0]
    assert x.shape == (8, NTOK, C), x.shape

    mt = np.float16 if _MODE == "fp16" else np.float32
    wqkT = np.ascontiguousarray(w_qkv[:2 * C].T.astype(mt))
    wvT = np.ascontiguousarray(w_qkv[2 * C:].T.astype(mt))
    wpT = np.ascontiguousarray(w_proj.T.astype(mt))
    bp = np.ascontiguousarray(b_proj.reshape(C, 1))
    in_maps = []
    for b in range(B):
        xT = np.zeros((C, T), mt)
        xT[:, :NTOK] = x[b].T.astype(mt)
        in_maps.append({"xT": xT, "wqkT": wqkT, "wvT": wvT, "wpT": wpT, "bp": bp})

    if _MODE not in _NC_CACHE:
        _NC_CACHE[_MODE] = build(matmul_dtype=_MODE)
    nc = _NC_CACHE[_MODE]
    from concourse import bass_utils
    res = bass_utils.run_bass_kernel_spmd(nc, in_maps, core_ids=list(range(B)),
                                          trace=_trace, tmpdir=_tmpdir)
    global LAST_RESULT
    LAST_RESULT = res
    y = np.stack([res.results[b]["yT"][:, :NTOK].T for b in range(B)])
    return np.ascontiguousarray(y.astype(np.float32))


# revision 50
# speedup vs baseline: 1.3244x; 1.2166x over previous
"""Trainium2 Bass kernel: batched multi-head self-attention (nn_Attention).

y = softmax(q k^T / sqrt(64)) v, projected; x (8, 1025, 768), 12 heads x 64.

Strategy: batch-parallel across the 8 NeuronCores (one batch element per
core, no collectives). Per core, everything is kept feature-major
(transposed) so no on-chip transposes are needed:
  qkT = wqkT.T @ xT;  v = xT.T @ wvT (with a per-head ones column);
  scoresT = kT.T @ qpad (keys on partitions; q zero-padded per head so the
  K stationary is the full 128-row head pair — full PE rows, no HAM droop);
  exp on the scalar engine;  [v|1](128-wide).T @ attnT accumulated over key
  tiles yields the weighted values AND the softmax denominator in one PSUM
  accumulation;  normalize via fast-approx reciprocal + gpsimd
  partition-broadcast;  yT = wpT.T @ aoT + bp.
Queries are processed in three 344-column chunks (no scalar tail pass).
Operands are fp16 (inputs/weights/q/k/v, ~2e-3 relative accuracy) except the
exp'd attention weights, which are bf16 (exp reaches ~5e6, beyond fp16
range); all accumulation is fp32 in PSUM.
"""
import sys

try:
    import concourse.bass  # noqa: F401
except ImportError:
    sys.path.insert(0, "/opt/trn_rl_repo")

import numpy as np

from contextlib import ExitStack

import concourse.bass as bass
import concourse.tile as tile
from concourse import bacc, mybir

F32 = mybir.dt.float32
F32R = mybir.dt.float32r
BF16 = mybir.dt.bfloat16
F16 = mybir.dt.float16

C = 768
H = 12
D = 64
NTOK = 1025
T = 1032
CT = C // 128
SCALE = D ** -0.5

KT = [(i * 128, 128) for i in range(8)] + [(1024, 8)]
QC = [(0, 344), (344, 344), (688, 344)]
PC = [(0, 512), (512, 512), (1024, 8)]
SC_GROUPS = [(0, 1), (2, 3), (4, 5), (6, 7), (8,)]
VW = 65


def build(matmul_dtype="fp16"):
    # MT: projection operands (x, weights, aoT). AT: q/k/v storage.
    # ATTN: exp output / AV moving operand (bf16: exp can reach ~5e6,
    # which overflows fp16).
    if matmul_dtype == "fp16":
        MT = AT = F16
        ATTN = BF16
    elif matmul_dtype == "bf16":
        MT = AT = ATTN = BF16
    else:
        MT = AT = ATTN = F32
    nc = bacc.Bacc("TRN2", target_bir_lowering=False, debug=False, num_devices=8)

    xT_d = nc.dram_tensor("xT", [C, T], MT, kind="ExternalInput")
    wqkT_d = nc.dram_tensor("wqkT", [C, 2 * C], MT, kind="ExternalInput")
    wvT_d = nc.dram_tensor("wvT", [C, C], MT, kind="ExternalInput")
    wpT_d = nc.dram_tensor("wpT", [C, C], MT, kind="ExternalInput")
    bp_d = nc.dram_tensor("bp", [C, 1], F32, kind="ExternalInput")
    yT_d = nc.dram_tensor("yT", [C, T], F16 if matmul_dtype == "fp16" else F32,
                          kind="ExternalOutput")

    with tile.TileContext(nc) as tc, ExitStack() as ctx:
        p_k = ctx.enter_context(tc.tile_pool(name="k", bufs=1))
        p_qp = ctx.enter_context(tc.tile_pool(name="qp", bufs=1))
        p_v = ctx.enter_context(tc.tile_pool(name="v", bufs=1))
        p_ao = ctx.enter_context(tc.tile_pool(name="ao", bufs=1))
        p_bp = ctx.enter_context(tc.tile_pool(name="bp", bufs=1))
        p_attn = ctx.enter_context(tc.tile_pool(name="attn", bufs=1))
        p_sm = ctx.enter_context(tc.tile_pool(name="sm", bufs=6))
        p_stage = ctx.enter_context(tc.tile_pool(name="stage", bufs=3))
        p_wp = ctx.enter_context(tc.tile_pool(name="wp", bufs=1))

        # kT tiles: rows = both heads of a pair (128); qpad: per pair,
        # [128, 2T]: cols 0:T = h0's q (rows 64:128 zero), cols T:2T = h1's
        # q (rows 0:64 zero) -> scores use the full-width K stationary.
        kT = [p_k.tile([128, T], AT, tag=f"kT{i}", name=f"kT{i}") for i in range(6)]
        qpad = [p_qp.tile([128, 2 * T], AT, tag=f"qp{i}", name=f"qp{i}")
                for i in range(6)]
        v_ext = [p_v.tile([128, H * VW + 63], AT, tag=f"v{i}", name=f"v{i}")
                 for i in range(9)]
        aoT = [p_ao.tile([128, T], MT, tag=f"ao{i}", name=f"ao{i}") for i in range(CT)]
        bp_sb = [p_bp.tile([128, 1], F32, tag=f"bp{i}", name=f"bp{i}")
                 for i in range(CT)]
        wp = [p_wp.tile([128, C], MT, tag=f"wp{i}", name=f"wp{i}") for i in range(CT)]

        with tc.tile_pool(name="x", bufs=1) as p_x, \
             tc.tile_pool(name="wv", bufs=1) as p_wv, \
             tc.tile_pool(name="psBC", bufs=6, space="PSUM") as psBC:
            xT = [p_x.tile([128, T], MT, tag=f"x{i}", name=f"x{i}") for i in range(CT)]
            wvT = [p_wv.tile([128, C], MT, tag=f"wv{i}", name=f"wv{i}")
                   for i in range(CT)]
            wqk = [p_wv.tile([128, 2 * C], MT, tag=f"wqk{i}", name=f"wqk{i}")
                   for i in range(CT)]
            # x/wv/wqk interleaved c-wise (consumed c-progressively);
            # wp/bp last -- not needed until the out-projection
            for c in range(CT):
                nc.sync.dma_start(xT[c][:], xT_d.ap()[c * 128:(c + 1) * 128, :])
                nc.sync.dma_start(wvT[c][:], wvT_d.ap()[c * 128:(c + 1) * 128, :])
            for c in range(CT):
                nc.sync.dma_start(wqk[c][:], wqkT_d.ap()[c * 128:(c + 1) * 128, :])
            for c in range(CT):
                nc.sync.dma_start(wp[c][:], wpT_d.ap()[c * 128:(c + 1) * 128, :])
                nc.sync.dma_start(bp_sb[c][:], bp_d.ap()[c * 128:(c + 1) * 128, :])

            # zero the cross-head quadrants of qpad (pad tokens come out
            # zero via the zeroed xT pad columns)
            for pair in range(6):
                _memset(nc, AT, qpad[pair][64:128, 0:T], one=False)
                _memset(nc, AT, qpad[pair][0:64, T:2 * T], one=False)

            # ---- phase C: v = xT.T @ wvT (token-major) ----
            with nc.named_scope("v_proj"):
                for nt, (noff, nsz) in enumerate(KT):
                    psa = psBC.tile([128, 512], F32, tag="proj", name="ps_proj")
                    psb = psBC.tile([128, 512], F32, tag="proj", name="ps_proj")
                    for c in range(CT):
                        xs = xT[c][:, noff:noff + nsz]
                        nc.tensor.matmul(psa[:nsz, :512], xs, wvT[c][:, 0:512],
                                         start=(c == 0), stop=(c == CT - 1))
                        nc.tensor.matmul(psb[:nsz, :256], xs, wvT[c][:, 512:768],
                                         start=(c == 0), stop=(c == CT - 1))
                    for (voff, vsz), ps in (((0, 512), psa), ((512, 256), psb)):
                        nh = vsz // D
                        h0 = voff // D
                        dst = (
                            v_ext[nt][0:nsz, h0 * VW:(h0 + nh) * VW]
                            .rearrange("p (hh w) -> p hh w", w=VW)[:, :, 0:D]
                        )
                        src = ps[0:nsz, 0:vsz].rearrange("p (hh w) -> p hh w", w=D)
                        nc.vector.tensor_copy(dst, src)
                    # ones column (valid tokens only) + zeroed pad/tail
                    if nt < 8:
                        ones_col = (
                            v_ext[nt][0:nsz, 0:H * VW]
                            .rearrange("p (hh w) -> p hh w", w=VW)[:, :, D:VW]
                        )
                        _memset(nc, AT, ones_col, one=True)
                    else:
                        pad_col = (
                            v_ext[nt][0:nsz, 0:H * VW]
                            .rearrange("p (hh w) -> p hh w", w=VW)[:, :, D:VW]
                        )
                        _memset(nc, AT, pad_col, one=False)
                        one_row = (
                            v_ext[nt][0:1, 0:H * VW]
                            .rearrange("p (hh w) -> p hh w", w=VW)[:, :, D:VW]
                        )
                        _memset(nc, AT, one_row, one=True)
                for nt in range(9):
                    _memset(nc, AT, v_ext[nt][:, H * VW:H * VW + 63], one=False)

            # ---- phase B: q/k projections ----
            with nc.named_scope("qk_proj"):
                for ot in range(12):
                    pss = [psBC.tile([128, 512], F32, tag="proj", name="ps_proj")
                           for _ in range(3)]
                    for c in range(CT):
                        w = wqk[c][:, ot * 128:(ot + 1) * 128]
                        for (qoff, qsz), ps in zip(PC, pss):
                            nc.tensor.matmul(ps[:, :qsz], w,
                                             xT[c][:, qoff:qoff + qsz],
                                             start=(c == 0), stop=(c == CT - 1))
                    if ot < 6:
                        # q: split the two heads into zero-padded halves
                        for (qoff, qsz), ps in zip(PC, pss):
                            nc.vector.tensor_copy(
                                qpad[ot][0:64, qoff:qoff + qsz], ps[0:64, :qsz])
                            nc.vector.tensor_copy(
                                qpad[ot][64:128, T + qoff:T + qoff + qsz],
                                ps[64:128, :qsz])
                    else:
                        # k casts ride the idle scalar engine
                        for (qoff, qsz), ps in zip(PC, pss):
                            nc.scalar.activation(
                                kT[ot - 6][:, qoff:qoff + qsz], ps[:, :qsz],
                                mybir.ActivationFunctionType.Copy)

        # ---- phase D: attention ----
        with nc.named_scope("attn"), \
             tc.tile_pool(name="psD", bufs=1, space="PSUM") as psD:

            def keep_warm(n):
                # dependency-free full-row matmuls that execute while the
                # final normalize drains, keeping the PE HAM clock hot
                ps = psD.tile([128, 512], F32, tag="av", name="ps_av", bufs=2)
                for _ in range(n):
                    nc.tensor.matmul(ps[:, 0:344], kT[0][:, 0:128],
                                     qpad[0][:, 0:344], start=True, stop=True)

            _emit_attention(nc, tc, kT, qpad, v_ext, aoT, psD, p_attn, p_sm,
                            ATTN, keep_warm)

        # ---- phase E: out projection ----
        with nc.named_scope("out_proj"), \
             tc.tile_pool(name="psE", bufs=6, space="PSUM") as psE:
            for ot in range(CT):
                pss = [psE.tile([128, 512], F32, tag="oproj", name="ps_oproj")
                       for _ in range(3)]
                for c in range(CT):
                    w = wp[c][:, ot * 128:(ot + 1) * 128]
                    for (qoff, qsz), ps in zip(QC, pss):
                        nc.tensor.matmul(ps[:, :qsz], w,
                                         aoT[c][:, qoff:qoff + qsz],
                                         start=(c == 0), stop=(c == CT - 1))
                st = p_stage.tile([128, T], F16 if MT == F16 else F32,
                                  tag="ystage", name="ystage", bufs=3)
                for (qoff, qsz), ps in zip(QC, pss):
                    nc.scalar.activation(st[:, qoff:qoff + qsz], ps[:, :qsz],
                                         mybir.ActivationFunctionType.Identity,
                                         bias=bp_sb[ot][:, 0:1])
                nc.sync.dma_start(yT_d.ap()[ot * 128:(ot + 1) * 128, :], st[:])

    nc.compile()
    return nc


def _memset(nc, AT, ap, one):
    if AT == BF16:
        nc.vector.memset(ap.bitcast(mybir.dt.uint16), 0x3F80 if one else 0)
    elif AT == F16:
        nc.vector.memset(ap.bitcast(mybir.dt.uint16), 0x3C00 if one else 0)
    else:
        nc.vector.memset(ap.bitcast(mybir.dt.uint32), 0x3F800000 if one else 0)


def _emit_attention(nc, tc, kT, qpad, v_ext, aoT, psD, p_attn, p_sm, ATTN,
                    keep_warm):
    def scores_mm(sc, pair, h_in_pair, kt, dst_off, qoff, qsz):
        koff, ksz = KT[kt]
        nc.tensor.matmul(
            sc[0:ksz, dst_off:dst_off + qsz],
            kT[pair][:, koff:koff + ksz],
            qpad[pair][:, h_in_pair * T + qoff:h_in_pair * T + qoff + qsz],
            start=True, stop=True,
        )

    def av_mm(av, h, kt, src, src_off, qsz):
        koff, ksz = KT[kt]
        nc.tensor.matmul(
            av[0:128, 0:qsz],
            v_ext[kt][0:ksz, h * VW:h * VW + 128],
            src[0:ksz, src_off:src_off + qsz],
            start=(kt == 0), stop=(kt == 8),
            skip_group_check=True,
        )

    def den_row(h):
        # halves live at partition bases 0 and 32 so the reciprocal APs
        # stay 32-aligned (engine ops reject unaligned partition bases)
        return h if h < 6 else h + 26

    pending = []

    def recip_half(den12, rec12, av_sbs, hs, qoff, qsz):
        r0 = den_row(hs[0])
        nc.vector.reciprocal(rec12[r0:r0 + len(hs), 0:qsz],
                             den12[r0:r0 + len(hs), 0:qsz])
        for h in hs:
            pending.append((h, rec12, av_sbs[h], qoff, qsz))

    def emit_muls(n):
        # drip-feed normalize multiplies so the DVE stream never bursts
        # (a burst delays av_sb copies -> AV PSUM slots release late ->
        # the PE stalls at chunk boundaries)
        for _ in range(min(n, len(pending))):
            h, rec12_, av_sb_, qoff_, qsz_ = pending.pop(0)
            rech = p_sm.tile([1, 352], F32, tag="rech", name="rech", bufs=13)
            nc.sync.dma_start(rech[0:1, 0:qsz_],
                              rec12_[den_row(h):den_row(h) + 1, 0:qsz_])
            bc = p_sm.tile([64, 512], F32, tag="bc", name="bc")
            nc.gpsimd.partition_broadcast(bc[0:64, 0:qsz_], rech[0:1, 0:qsz_])
            p0 = 64 * (h % 2)
            nc.vector.tensor_mul(
                aoT[h // 2][p0:p0 + 64, qoff_:qoff_ + qsz_],
                av_sb_[0:D, 0:qsz_],
                bc[0:64, 0:qsz_],
            )

    prev_chunk = None
    for ci, (qoff, qsz) in enumerate(QC):
        # batched denominator reciprocal, in two halves: denom rows are
        # DMA-gathered into den12; heads 0-5 reciprocal after pair 2 (their
        # normalize overlaps pairs 3-5), heads 6-11 after pair 5
        den12 = p_sm.tile([66, 352], F32, tag="den12", name="den12", bufs=2)
        rec12 = p_sm.tile([66, 352], F32, tag="rec12", name="rec12", bufs=2)
        av_sbs = {}
        for pair in range(6):
            h0 = 2 * pair
            h1 = 2 * pair + 1
            avs = {h0: psD.tile([128, 512], F32, tag="av", name="ps_av", bufs=2),
                   h1: psD.tile([128, 512], F32, tag="av", name="ps_av", bufs=2)}
            for g in SC_GROUPS:
                sc_slots = {}
                for h_in_pair in range(2):
                    sc_slots[h_in_pair] = psD.tile([128, 1024], F32, tag="sc",
                                                   name="ps_sc", bufs=3)
                for gi, kt in enumerate(g):
                    for h_in_pair in range(2):
                        scores_mm(sc_slots[h_in_pair], pair, h_in_pair, kt,
                                  gi * 512, qoff, qsz)
                at = {}
                for h_in_pair in range(2):
                    h = 2 * pair + h_in_pair
                    a = p_attn.tile([128, 1024], ATTN, tag="attnT", name="attnT",
                                    bufs=8)
                    at[h] = a
                    pmax = max(KT[kt][1] for kt in g)
                    if len(g) == 2:
                        # src: the two kt blocks sit at bank-aligned cols
                        # 0 and 512; dst is packed at stride qsz
                        src = sc_slots[h_in_pair][0:pmax, 0:1024] \
                            .rearrange("p (g w) -> p g w", w=512)[:, :, 0:qsz]
                        dst = a[0:pmax, 0:2 * qsz] \
                            .rearrange("p (g w) -> p g w", w=qsz)
                    else:
                        src = sc_slots[h_in_pair][0:pmax, 0:qsz]
                        dst = a[0:pmax, 0:qsz]
                    nc.scalar.activation(
                        dst, src, mybir.ActivationFunctionType.Exp, scale=SCALE,
                    )
                for gi, kt in enumerate(g):
                    av_mm(avs[h0], h0, kt, at[h0], gi * qsz, qsz)
                    av_mm(avs[h1], h1, kt, at[h1], gi * qsz, qsz)
            # stage av (+denominator row) out of PSUM; gather denom rows
            for h in (h0, h1):
                av_sb = p_sm.tile([VW, 352], F32, tag="avsb", name="avsb",
                                  bufs=21)
                nc.vector.tensor_copy(av_sb[0:VW, 0:qsz], avs[h][0:VW, 0:qsz])
                nc.sync.dma_start(den12[den_row(h):den_row(h) + 1, 0:qsz],
                                  av_sb[D:VW, 0:qsz])
                av_sbs[h] = av_sb
            emit_muls(2)
            if pair == 2:
                recip_half(den12, rec12, av_sbs, range(0, 6), qoff, qsz)
        recip_half(den12, rec12, av_sbs, range(6, 12), qoff, qsz)
        prev_chunk = (qoff, qsz)
    emit_muls(len(pending))
    keep_warm(10)


_NC_CACHE = {}
_MODE = "fp16"
LAST_RESULT = None


def kernel(x, w_qkv, w_proj, b_proj, *, _trace=False, _tmpdir=None):
    x = np.asarray(x, np.float32)
    w_qkv = np.asarray(w_qkv, np.float32)
    w_proj = np.asarray(w_proj, np.float32)
    b_proj = np.asarray(b_proj, np.float32)
    B = x.shape[0]
    assert x.shape == (8, NTOK, C), x.shape

    mt = np.float16 if _MODE == "fp16" else np.float32
    wqkT = np.ascontiguousarray(w_qkv[:2 * C].T.astype(mt))
    wvT = np.ascontiguousarray(w_qkv[2 * C:].T.astype(mt))
    wpT = np.ascontiguousarray(w_proj.T.astype(mt))
    bp = np.ascontiguousarray(b_proj.reshape(C, 1))
    in_maps = []
    for b in range(B):
        xT = np.zeros((C, T), mt)
        xT[:, :NTOK] = x[b].T.astype(mt)
        in_maps.append({"xT": xT, "wqkT": wqkT, "wvT": wvT, "wpT": wpT, "bp": bp})

    if _MODE not in _NC_CACHE:
        _NC_CACHE[_MODE] = build(matmul_dtype=_MODE)
    nc = _NC_CACHE[_MODE]
    from concourse import bass_utils
    res = bass_utils.run_bass_kernel_spmd(nc, in_maps, core_ids=list(range(B)),
                                          trace=_trace, tmpdir=_tmpdir)
    global LAST_RESULT
    LAST_RESULT = res
    y = np.stack([res.results[b]["yT"][:, :NTOK].T for b in range(B)])
    return np.ascontiguousarray(y.astype(np.float32))
